# revision 13
# baseline (speedup 1.0000x reference)
"""Trainium2 Bass kernel for nn_DecoderSmoothedMaxPoolingLoss.

Loss (see reference):
  neg  = -log(1 - X)                                    (B,T,K)
  loss = sum_{b, t<len_b, k} neg
         - sum_{b, i in [0,Lw_b), k=tgt_b} neg[b, tau_s_b + i, k]
         + sum_b -log( max_j  clip(conv_same(win_b * valid_b, filt), EPS, 1) * valid_b )
  where tau_s = max(0, w_end + 40 - 60), tau_e = min(tau_s + 60, len),
  Lw = tau_e - tau_s, win_b[i] = X[b, tau_s_b + i, tgt_b].

Sharding: pure data parallel over batch - 8 batches per core on 8 cores.
Each core computes its partial scalar loss on device; host sums the 8
partials (the "all-reduce").

Per core (8 local batches = 12.8 MB), v2 layout:
  slab:     local X viewed flat as (128, 25000): partition p holds flat
            [p*25000, (p+1)*25000). 400000 elems/batch = exactly 16
            partition rows per batch, so the t<len mask is per-partition
            except for ONE partial row per batch. Loaded as NCH chunks
            of (128, S) - 12.5KB+ descriptors per partition - split
            across the HWDGE (sync) and SWDGE (gpsimd) queues.
  big term: ONE ACT instruction per chunk:
            activation(Ln, bias=1, scale=scl[:,c], accum_out=C[:,c])
            computes ln(1 - x) AND the per-partition row sum. scl is a
            host-built per-(row,chunk) mask in {-1, 0}: rows/chunks with
            any invalid element get scale 0 (ln(1) = 0 contribution).
  boundary: per batch, the one chunk that straddles t=len_b is dropped
            by scl and its valid prefix (< S elems) is re-gathered via
            indirect DMA into G (24, 1600) (3 rows of 1600 per batch,
            1600-aligned), Ln'd, and folded with a host mask in {-1,0}
            via ONE fused tensor_tensor_reduce.
  windows:  indirect DMA gathers 60 contiguous t-rows per batch into
            Wp (8, 6000); one-hot select of k = tgt via host-built
            ohrep -> win_raw (8,60).  Exclusion = sum valid*ln(1-win)
            (fused TTR).  Positive term: conv as two small matmuls,
            clip, mask, row-max, Ln.
  final:    partition sums via matmuls with (-1/+1) vectors into one
            PSUM row, free reduce -> scalar out.
"""

import numpy as np

import concourse.bass as bass
import concourse.tile as tile
from concourse import bacc
from concourse import mybir
from concourse import bass_utils
from concourse.bass import IndirectOffsetOnAxis

AF = mybir.ActivationFunctionType
ALU = mybir.AluOpType
AX = mybir.AxisListType
FP = mybir.dt.float32
I32 = mybir.dt.int32

B, T, K = 64, 4000, 100
WIN, OFFSET_D, TRUNC, SIGMA = 60, 40, 21, 9
EPS = 1e-8
NCORES = 8
BLOC = B // NCORES          # 8 batches per core
NP = 128                    # slab partitions
FROW = 25000                # elems per partition row (= T*K/16)
RPB = 16                    # partition rows per batch
NCH = 8                     # free-axis chunks
S = FROW // NCH             # 3125 elems per chunk per partition
NELEM = NP * FROW           # 3.2M valid elems per core
GF = 1600                   # correction gather row width
GR = 3                      # correction rows per batch
GROWS = GR * BLOC           # 24
PADDED = (NELEM // GF + 1) * GF  # 3201600: covers worst-case gather

WA = WIN * K + WIN + BLOC   # auxA (8, 6068): ohrep | valid8 | I8


def _filt_np():
    half = TRUNC // 2
    x = np.arange(-half, half + 1, dtype=np.float32)
    g = np.exp(-0.5 * (x / SIGMA) ** 2).astype(np.float32)
    g = g / g.sum()
    f = np.zeros(WIN, np.float32)
    c = WIN // 2
    f[c - half:c + half + 1] = g
    return f


def _conv_matrix():
    # smoothed[j] = sum_i win[i] * filt[i - j + pl], pl = (WIN-1)//2
    f = _filt_np()
    pl = (WIN - 1) // 2
    idx = np.arange(WIN)
    u = idx[:, None] - idx[None, :] + pl          # (i, j)
    M = np.where((u >= 0) & (u < WIN), f[np.clip(u, 0, WIN - 1)], 0.0)
    return M.astype(np.float32)


_NC_CACHE = None


def _build_program():
    global _NC_CACHE
    if _NC_CACHE is not None:
        return _NC_CACHE

    nc = bacc.Bacc("TRN2", debug=False)
    # Xs: chunk-major permuted slab (each chunk a contiguous NP*S block);
    # Xg: original layout (padded) for the indirect gathers.
    Xs = nc.dram_tensor("Xs", [NELEM], FP, kind="ExternalInput").ap()
    Xg = nc.dram_tensor("Xg", [PADDED], FP, kind="ExternalInput").ap()
    gofs = nc.dram_tensor("gofs", [BLOC, 1], I32, kind="ExternalInput").ap()
    coff = nc.dram_tensor("coff", [GROWS, 1], I32, kind="ExternalInput").ap()
    auxA = nc.dram_tensor("auxA", [BLOC, WA], FP, kind="ExternalInput").ap()
    auxM = nc.dram_tensor("auxM", [WIN, WIN], FP, kind="ExternalInput").ap()
    mcorr = nc.dram_tensor("mcorr", [GROWS, GF], FP, kind="ExternalInput").ap()
    scl = nc.dram_tensor("scl", [NP, NCH], FP, kind="ExternalInput").ap()
    outd = nc.dram_tensor("out", [1, 1], FP, kind="ExternalOutput").ap()

    with tile.TileContext(nc) as tc:
        with tc.tile_pool(name="xin", bufs=NCH) as xin_pool, \
             tc.tile_pool(name="small", bufs=1) as small, \
             tc.tile_pool(name="psum", bufs=1, space="PSUM") as psum:

            # ---------- big slab chunks, alternating DMA queues ----------
            # chunk c is the contiguous block [c*NP*S, (c+1)*NP*S) of Xs.
            CH = NP * S

            def chunk_dma(eng, xb, c):
                eng.dma_start(
                    out=xb[:],
                    in_=Xs[c * CH:(c + 1) * CH].rearrange(
                        "(p f) -> p f", p=NP))

            xtiles = [xin_pool.tile([NP, S], FP, tag="xb", name=f"xb{c}")
                      for c in range(NCH)]
            # gpsimd (SWDGE) queue: offsets first (tiny, same-queue so no
            # cross-engine wait), two big chunks, then the indirect
            # gathers (Q7 stalls on the offset data, but the ring already
            # holds 3.2MB of chunk descriptors), then remaining chunks.
            gofs_sb = small.tile([BLOC, 1], I32)
            nc.gpsimd.dma_start(out=gofs_sb[:], in_=gofs)
            coff_sb = small.tile([GROWS, 1], I32)
            nc.gpsimd.dma_start(out=coff_sb[:], in_=coff)
            chunk_dma(nc.gpsimd, xtiles[1], 1)
            chunk_dma(nc.gpsimd, xtiles[3], 3)
            Wp = small.tile([BLOC, WIN * K], FP)
            nc.gpsimd.indirect_dma_start(
                out=Wp[:],
                out_offset=None,
                in_=Xg[0:NELEM].rearrange("(r k) -> r k", k=K),
                in_offset=IndirectOffsetOnAxis(ap=gofs_sb[:, :1], axis=0),
            )
            G = small.tile([GROWS, GF], FP)
            nc.gpsimd.indirect_dma_start(
                out=G[:],
                out_offset=None,
                in_=Xg.rearrange("(r f) -> r f", f=GF),
                in_offset=IndirectOffsetOnAxis(ap=coff_sb[:, :1], axis=0),
            )
            chunk_dma(nc.gpsimd, xtiles[5], 5)
            chunk_dma(nc.gpsimd, xtiles[7], 7)

            # sync (HWDGE) queue: pure big chunks
            for c in range(0, NCH, 2):
                chunk_dma(nc.sync, xtiles[c], c)

            # scalar (HWDGE) queue: small aux loads, dispatched from the
            # ACT stream before any ACT compute
            scl_sb = small.tile([NP, NCH], FP)
            nc.scalar.dma_start(out=scl_sb[:], in_=scl)
            auxA_sb = small.tile([BLOC, WA], FP)
            nc.scalar.dma_start(out=auxA_sb[:], in_=auxA)
            auxM_sb = small.tile([WIN, WIN], FP)
            nc.scalar.dma_start(out=auxM_sb[:], in_=auxM)
            mcorr_sb = small.tile([GROWS, GF], FP)
            nc.scalar.dma_start(out=mcorr_sb[:], in_=mcorr)

            ohrep_sl = auxA_sb[0:BLOC, 0:WIN * K]
            valid_sl = auxA_sb[0:BLOC, WIN * K:WIN * K + WIN]
            I8_sl = auxA_sb[0:BLOC, WIN * K + WIN:WA]

            C = small.tile([NP, NCH], FP)
            nc.vector.memset(C[:], 0.0)

            # ---------- correction path Ln first (data arrives early) ----
            lnG = small.tile([GROWS, GF], FP)
            nc.scalar.activation(out=lnG[:], in_=G[:], func=AF.Ln,
                                 bias=1.0, scale=-1.0)
            gcol = small.tile([GROWS, 1], FP)
            nc.vector.tensor_tensor(out=lnG[:], in0=lnG[:], in1=mcorr_sb[:],
                                    op=ALU.mult)
            nc.vector.tensor_reduce(out=gcol[:], in_=lnG[:], axis=AX.X,
                                    op=ALU.add)

            # ---------- big term: ONE fused ACT op per chunk ----------
            def chunk_act(c):
                xb = xtiles[c]
                nc.scalar.activation(out=xb[:], in_=xb[:], func=AF.Ln,
                                     bias=1.0, scale=scl_sb[:, c:c + 1],
                                     accum_out=C[:, c:c + 1])

            for c in range(NCH - 2):
                chunk_act(c)

            # ---------- window path (ACT ops land before last chunks) ----
            nc.vector.tensor_tensor(out=Wp[:], in0=Wp[:], in1=ohrep_sl,
                                    op=ALU.mult)
            win_raw = small.tile([BLOC, WIN], FP)
            nc.vector.tensor_reduce(
                out=win_raw[:],
                in_=Wp[:].rearrange("b (i k) -> b i k", k=K),
                axis=AX.X, op=ALU.add)
            # positive term part 1: win_v = win_raw * valid
            winv = small.tile([BLOC, WIN], FP)
            nc.vector.tensor_tensor(out=winv[:], in0=win_raw[:],
                                    in1=valid_sl, op=ALU.mult)
            # exclusion: + sum_i valid * ln(1 - win_raw), fused
            expcol = small.tile([BLOC, 2], FP)
            lnw = small.tile([BLOC, WIN], FP)
            nc.scalar.activation(out=lnw[:], in_=win_raw[:], func=AF.Ln,
                                 bias=1.0, scale=-1.0)
            lnwv = small.tile([BLOC, WIN], FP)
            nc.vector.tensor_tensor(out=lnwv[:], in0=lnw[:], in1=valid_sl,
                                    op=ALU.mult)
            nc.vector.tensor_reduce(out=expcol[:, 0:1], in_=lnwv[:],
                                    axis=AX.X, op=ALU.add)
            # smoothed = win_v @ M (transpose first via identity)
            wvt_ps = psum.tile([WIN, BLOC], FP)
            nc.tensor.matmul(out=wvt_ps[:], lhsT=winv[:], rhs=I8_sl,
                             start=True, stop=True)
            wvt = small.tile([WIN, BLOC], FP)
            nc.vector.tensor_copy(out=wvt[:], in_=wvt_ps[:])
            sm_ps = psum.tile([BLOC, WIN], FP)
            nc.tensor.matmul(out=sm_ps[:], lhsT=wvt[:], rhs=auxM_sb[:],
                             start=True, stop=True)
            smc = small.tile([BLOC, WIN], FP)
            nc.vector.tensor_scalar(out=smc[:], in0=sm_ps[:],
                                    scalar1=EPS, scalar2=1.0,
                                    op0=ALU.max, op1=ALU.min)
            smv = small.tile([BLOC, WIN], FP)
            nc.vector.tensor_tensor(out=smv[:], in0=smc[:], in1=valid_sl,
                                    op=ALU.mult)
            mx = small.tile([BLOC, 1], FP)
            nc.vector.tensor_reduce(out=mx[:], in_=smv[:], axis=AX.X,
                                    op=ALU.max)
            lnmx = small.tile([BLOC, 1], FP)
            nc.scalar.activation(out=lnmx[:], in_=mx[:], func=AF.Ln)
            nc.vector.tensor_scalar_mul(expcol[:, 1:2], lnmx[:], -1.0)

            # last two chunks after the window-path ACT ops
            for c in range(NCH - 2, NCH):
                chunk_act(c)

            # ---------- final partition reduce ----------
            negones = small.tile([NP, 1], FP)
            nc.vector.memset(negones[:], -1.0)
            ones24 = small.tile([GROWS, 1], FP)
            nc.vector.memset(ones24[:], 1.0)
            tot_ps = psum.tile([1, NCH + 3], FP)
            nc.tensor.matmul(out=tot_ps[:, 0:NCH], lhsT=negones[:], rhs=C[:],
                             start=True, stop=True)
            nc.tensor.matmul(out=tot_ps[:, NCH:NCH + 1],
                             lhsT=ones24[:], rhs=gcol[:],
                             start=True, stop=True)
            nc.tensor.matmul(out=tot_ps[:, NCH + 1:NCH + 3],
                             lhsT=ones24[0:BLOC, :], rhs=expcol[:],
                             start=True, stop=True)
            tot = small.tile([1, 1], FP)
            nc.vector.tensor_reduce(out=tot[:], in_=tot_ps[:], axis=AX.X,
                                    op=ALU.add)
            nc.sync.dma_start(out=outd, in_=tot[:])

    nc.compile()
    _NC_CACHE = nc
    return nc


def _make_in_maps(X, lengths, tgt, w_end):
    X = np.ascontiguousarray(np.asarray(X, dtype=np.float32))
    lengths = np.asarray(lengths, dtype=np.int64)
    tgt = np.asarray(tgt, dtype=np.int64)
    w_end = np.asarray(w_end, dtype=np.int64)

    tau_s = np.maximum(0, w_end + OFFSET_D - WIN)
    tau_e = np.minimum(tau_s + WIN, lengths)
    Lw = tau_e - tau_s

    Mmat = _conv_matrix()
    I8 = np.eye(BLOC, dtype=np.float32)

    in_maps = []
    for cr in range(NCORES):
        bs = slice(cr * BLOC, (cr + 1) * BLOC)
        ls, ts, lw, tg = lengths[bs], tau_s[bs], Lw[bs], tgt[bs]

        oh = np.zeros((BLOC, K), np.float32)
        oh[np.arange(BLOC), tg] = 1.0
        ohrep = np.broadcast_to(oh[:, None, :], (BLOC, WIN, K)) \
            .reshape(BLOC, WIN * K)
        valid8 = (np.arange(WIN)[None, :] < lw[:, None]).astype(np.float32)
        auxA = np.concatenate([ohrep, valid8, I8], axis=1)  # (8, WA)

        # per-partition-row valid-element counts: row p of batch b covers
        # flat [p*FROW, (p+1)*FROW) within the batch -> thr elems valid
        r = np.arange(NP)
        thr = np.clip(ls[r // RPB] * K - (r % RPB) * FROW, 0, FROW)  # (128,)
        # scl[p, c] = -1 if chunk c of row p is fully valid else 0
        cidx = np.arange(NCH)
        scl_arr = np.where(thr[:, None] >= (cidx[None, :] + 1) * S,
                           np.float32(-1.0), np.float32(0.0))

        # boundary corrections: per batch at most one row with 0<thr<FROW;
        # its partial chunk [c0*S, thr) is re-gathered 1600-aligned.
        coff_arr = np.zeros((GROWS, 1), np.int32)
        mcorr_arr = np.zeros((GROWS, GF), np.float32)
        for b in range(BLOC):
            rows = np.where((thr > 0) & (thr < FROW)
                            & (r // RPB == b))[0]
            if len(rows) == 0:
                continue
            p0 = int(rows[0])
            th = int(thr[p0])
            c0 = th // S
            L = th - c0 * S
            if L == 0:
                continue
            g = p0 * FROW + c0 * S          # global start elem of partial
            a = (g // GF) * GF              # aligned gather start
            for u in range(GR):
                coff_arr[GR * b + u, 0] = a // GF + u
                e = a + GF * u + np.arange(GF)      # global elem idx
                mcorr_arr[GR * b + u] = np.where(
                    (e >= g) & (e < g + L), np.float32(-1.0),
                    np.float32(0.0))

        gofs_arr = (np.arange(BLOC) * T + ts).astype(np.int32) \
            .reshape(BLOC, 1)

        Xp = np.zeros(PADDED, np.float32)
        Xp[:NELEM] = X[bs].ravel()
        # chunk-major permutation: chunk c of the (NP, FROW) slab becomes
        # the contiguous block [c*NP*S, (c+1)*NP*S)
        Xcm = np.ascontiguousarray(
            Xp[:NELEM].reshape(NP, NCH, S).transpose(1, 0, 2)).ravel()
        in_maps.append({
            "Xs": Xcm,
            "Xg": Xp,
            "gofs": gofs_arr,
            "coff": coff_arr,
            "auxA": np.ascontiguousarray(auxA),
            "auxM": np.ascontiguousarray(Mmat),
            "mcorr": mcorr_arr,
            "scl": np.ascontiguousarray(scl_arr),
        })
    return in_maps


def kernel(X, lengths, tgt, w_end):
    nc = _build_program()
    in_maps = _make_in_maps(X, lengths, tgt, w_end)
    res = bass_utils.run_bass_kernel_spmd(
        nc, in_maps, core_ids=list(range(NCORES)))
    total = np.float32(0.0)
    for c in range(NCORES):
        total += np.float32(res.results[c]["out"][0, 0])
    return np.array(total, dtype=np.float32)


# revision 17
# speedup vs baseline: 1.1182x; 1.1182x over previous
"""Trainium2 Bass kernel for nn_DecoderSmoothedMaxPoolingLoss.

Loss (see reference):
  neg  = -log(1 - X)                                    (B,T,K)
  loss = sum_{b, t<len_b, k} neg
         - sum_{b, i in [0,Lw_b), k=tgt_b} neg[b, tau_s_b + i, k]
         + sum_b -log( max_j  clip(conv_same(win_b * valid_b, filt), EPS, 1) * valid_b )
  where tau_s = max(0, w_end + 40 - 60), tau_e = min(tau_s + 60, len),
  Lw = tau_e - tau_s, win_b[i] = X[b, tau_s_b + i, tgt_b].

Sharding: pure data parallel over batch - 8 batches per core on 8 cores.
Each core computes its partial scalar loss on device; host sums the 8
partials (the "all-reduce").

Per core (8 local batches = 12.8 MB), v2 layout:
  slab:     local X viewed flat as (128, 25000): partition p holds flat
            [p*25000, (p+1)*25000). 400000 elems/batch = exactly 16
            partition rows per batch, so the t<len mask is per-partition
            except for ONE partial row per batch. Loaded as NCH chunks
            of (128, S) - 12.5KB+ descriptors per partition - split
            across the HWDGE (sync) and SWDGE (gpsimd) queues.
  big term: ONE ACT instruction per chunk:
            activation(Ln, bias=1, scale=scl[:,c], accum_out=C[:,c])
            computes ln(1 - x) AND the per-partition row sum. scl is a
            host-built per-(row,chunk) mask in {-1, 0}: rows/chunks with
            any invalid element get scale 0 (ln(1) = 0 contribution).
  boundary: per batch, the one chunk that straddles t=len_b is dropped
            by scl and its valid prefix (< S elems) is re-gathered via
            indirect DMA into G (24, 1600) (3 rows of 1600 per batch,
            1600-aligned), Ln'd, and folded with a host mask in {-1,0}
            via ONE fused tensor_tensor_reduce.
  windows:  indirect DMA gathers 60 contiguous t-rows per batch into
            Wp (8, 6000); one-hot select of k = tgt via host-built
            ohrep -> win_raw (8,60).  Exclusion = sum valid*ln(1-win)
            (fused TTR).  Positive term: conv as two small matmuls,
            clip, mask, row-max, Ln.
  final:    partition sums via matmuls with (-1/+1) vectors into one
            PSUM row, free reduce -> scalar out.
"""

import numpy as np

import concourse.bass as bass
import concourse.tile as tile
from concourse import bacc
from concourse import mybir
from concourse import bass_utils
from concourse.bass import IndirectOffsetOnAxis

AF = mybir.ActivationFunctionType
ALU = mybir.AluOpType
AX = mybir.AxisListType
FP = mybir.dt.float32
I32 = mybir.dt.int32

B, T, K = 64, 4000, 100
WIN, OFFSET_D, TRUNC, SIGMA = 60, 40, 21, 9
EPS = 1e-8
NCORES = 8
BLOC = B // NCORES          # 8 batches per core
NP = 128                    # slab partitions
FROW = 25000                # elems per partition row (= T*K/16)
RPB = 16                    # partition rows per batch
NCH = 8                     # free-axis chunks
S = FROW // NCH             # 3125 elems per chunk per partition
NELEM = NP * FROW           # 3.2M valid elems per core
GF = 1600                   # correction gather row width
GR = 3                      # correction rows per batch
GROWS = GR * BLOC           # 24
PADDED = (NELEM // GF + 1) * GF  # 3201600: covers worst-case gather

WA = WIN * K + WIN + BLOC   # auxA (8, 6068): ohrep | valid8 | I8


def _filt_np():
    half = TRUNC // 2
    x = np.arange(-half, half + 1, dtype=np.float32)
    g = np.exp(-0.5 * (x / SIGMA) ** 2).astype(np.float32)
    g = g / g.sum()
    f = np.zeros(WIN, np.float32)
    c = WIN // 2
    f[c - half:c + half + 1] = g
    return f


def _conv_matrix():
    # smoothed[j] = sum_i win[i] * filt[i - j + pl], pl = (WIN-1)//2
    f = _filt_np()
    pl = (WIN - 1) // 2
    idx = np.arange(WIN)
    u = idx[:, None] - idx[None, :] + pl          # (i, j)
    M = np.where((u >= 0) & (u < WIN), f[np.clip(u, 0, WIN - 1)], 0.0)
    return M.astype(np.float32)


_NC_CACHE = None


def _build_program():
    global _NC_CACHE
    if _NC_CACHE is not None:
        return _NC_CACHE

    nc = bacc.Bacc("TRN2", debug=False)
    # Xs: chunk-major permuted slab (each chunk a contiguous NP*S block);
    # Xg: original layout (padded) for the indirect gathers.
    Xs = nc.dram_tensor("Xs", [NELEM], FP, kind="ExternalInput").ap()
    Xg = nc.dram_tensor("Xg", [PADDED], FP, kind="ExternalInput").ap()
    gofs = nc.dram_tensor("gofs", [BLOC, 1], I32, kind="ExternalInput").ap()
    coff = nc.dram_tensor("coff", [GROWS, 1], I32, kind="ExternalInput").ap()
    auxA = nc.dram_tensor("auxA", [BLOC, WA], FP, kind="ExternalInput").ap()
    auxM = nc.dram_tensor("auxM", [WIN, WIN], FP, kind="ExternalInput").ap()
    mcorr = nc.dram_tensor("mcorr", [GROWS, GF], FP, kind="ExternalInput").ap()
    scl = nc.dram_tensor("scl", [NP, NCH], FP, kind="ExternalInput").ap()
    outd = nc.dram_tensor("out", [1, 1], FP, kind="ExternalOutput").ap()

    with tile.TileContext(nc) as tc:
        with tc.tile_pool(name="xin", bufs=NCH) as xin_pool, \
             tc.tile_pool(name="small", bufs=1) as small, \
             tc.tile_pool(name="psum", bufs=1, space="PSUM") as psum:

            # ---------- big slab chunks, alternating DMA queues ----------
            # chunk c is the contiguous block [c*NP*S, (c+1)*NP*S) of Xs.
            CH = NP * S

            def chunk_dma(eng, xb, c):
                eng.dma_start(
                    out=xb[:],
                    in_=Xs[c * CH:(c + 1) * CH].rearrange(
                        "(p f) -> p f", p=NP))

            xtiles = [xin_pool.tile([NP, S], FP, tag="xb", name=f"xb{c}")
                      for c in range(NCH)]
            # gpsimd (SWDGE) queue: ALL small loads first (tiny, and the
            # gathers' offset inputs stay same-queue so no cross-engine
            # wait), two big chunks, then the indirect gathers (Q7 stalls
            # on the offset data, but the ring already holds 3.2MB of
            # chunk descriptors), then remaining chunks.
            gofs_sb = small.tile([BLOC, 1], I32)
            nc.gpsimd.dma_start(out=gofs_sb[:], in_=gofs)
            coff_sb = small.tile([GROWS, 1], I32)
            nc.gpsimd.dma_start(out=coff_sb[:], in_=coff)
            scl_sb = small.tile([NP, NCH], FP)
            nc.gpsimd.dma_start(out=scl_sb[:], in_=scl)
            auxA_sb = small.tile([BLOC, WA], FP)
            nc.gpsimd.dma_start(out=auxA_sb[:], in_=auxA)
            auxM_sb = small.tile([WIN, WIN], FP)
            nc.gpsimd.dma_start(out=auxM_sb[:], in_=auxM)
            mcorr_sb = small.tile([GROWS, GF], FP)
            nc.gpsimd.dma_start(out=mcorr_sb[:], in_=mcorr)
            chunk_dma(nc.gpsimd, xtiles[1], 1)
            chunk_dma(nc.gpsimd, xtiles[3], 3)
            Wp = small.tile([BLOC, WIN * K], FP)
            nc.gpsimd.indirect_dma_start(
                out=Wp[:],
                out_offset=None,
                in_=Xg[0:NELEM].rearrange("(r k) -> r k", k=K),
                in_offset=IndirectOffsetOnAxis(ap=gofs_sb[:, :1], axis=0),
            )
            G = small.tile([GROWS, GF], FP)
            nc.gpsimd.indirect_dma_start(
                out=G[:],
                out_offset=None,
                in_=Xg.rearrange("(r f) -> r f", f=GF),
                in_offset=IndirectOffsetOnAxis(ap=coff_sb[:, :1], axis=0),
            )
            chunk_dma(nc.gpsimd, xtiles[5], 5)
            chunk_dma(nc.gpsimd, xtiles[7], 7)

            # sync (HWDGE) queue: pure big chunks
            for c in range(0, NCH, 2):
                chunk_dma(nc.sync, xtiles[c], c)

            ohrep_sl = auxA_sb[0:BLOC, 0:WIN * K]
            valid_sl = auxA_sb[0:BLOC, WIN * K:WIN * K + WIN]
            I8_sl = auxA_sb[0:BLOC, WIN * K + WIN:WA]

            C = small.tile([NP, NCH], FP)
            nc.vector.memset(C[:], 0.0)

            # ---------- correction path Ln first (data arrives early);
            # its DVE ops are emitted after the window chain ----------
            lnG = small.tile([GROWS, GF], FP)
            nc.scalar.activation(out=lnG[:], in_=G[:], func=AF.Ln,
                                 bias=1.0, scale=-1.0)

            # ---------- big term: ONE fused ACT op per chunk ----------
            def chunk_act(c):
                xb = xtiles[c]
                nc.scalar.activation(out=xb[:], in_=xb[:], func=AF.Ln,
                                     bias=1.0, scale=scl_sb[:, c:c + 1],
                                     accum_out=C[:, c:c + 1])

            for c in range(NCH - 2):
                chunk_act(c)

            # ---------- window path (ACT ops land before last chunks) ----
            nc.vector.tensor_tensor(out=Wp[:], in0=Wp[:], in1=ohrep_sl,
                                    op=ALU.mult)
            win_raw = small.tile([BLOC, WIN], FP)
            nc.vector.tensor_reduce(
                out=win_raw[:],
                in_=Wp[:].rearrange("b (i k) -> b i k", k=K),
                axis=AX.X, op=ALU.add)
            # positive term part 1: win_v = win_raw * valid
            winv = small.tile([BLOC, WIN], FP)
            nc.vector.tensor_tensor(out=winv[:], in0=win_raw[:],
                                    in1=valid_sl, op=ALU.mult)
            # exclusion: + sum_i valid * ln(1 - win_raw), fused
            expcol = small.tile([BLOC, 2], FP)
            lnw = small.tile([BLOC, WIN], FP)
            nc.scalar.activation(out=lnw[:], in_=win_raw[:], func=AF.Ln,
                                 bias=1.0, scale=-1.0)
            lnwv = small.tile([BLOC, WIN], FP)
            nc.vector.tensor_tensor(out=lnwv[:], in0=lnw[:], in1=valid_sl,
                                    op=ALU.mult)
            nc.vector.tensor_reduce(out=expcol[:, 0:1], in_=lnwv[:],
                                    axis=AX.X, op=ALU.add)
            # smoothed = win_v @ M (transpose first via identity)
            wvt_ps = psum.tile([WIN, BLOC], FP)
            nc.tensor.matmul(out=wvt_ps[:], lhsT=winv[:], rhs=I8_sl,
                             start=True, stop=True)
            wvt = small.tile([WIN, BLOC], FP)
            nc.vector.tensor_copy(out=wvt[:], in_=wvt_ps[:])
            sm_ps = psum.tile([BLOC, WIN], FP)
            nc.tensor.matmul(out=sm_ps[:], lhsT=wvt[:], rhs=auxM_sb[:],
                             start=True, stop=True)
            smc = small.tile([BLOC, WIN], FP)
            nc.vector.tensor_scalar(out=smc[:], in0=sm_ps[:],
                                    scalar1=EPS, scalar2=1.0,
                                    op0=ALU.max, op1=ALU.min)
            smv = small.tile([BLOC, WIN], FP)
            nc.vector.tensor_tensor(out=smv[:], in0=smc[:], in1=valid_sl,
                                    op=ALU.mult)
            mx = small.tile([BLOC, 1], FP)
            nc.vector.tensor_reduce(out=mx[:], in_=smv[:], axis=AX.X,
                                    op=ALU.max)
            # correction fold (DVE) after the window chain
            gcol = small.tile([GROWS, 1], FP)
            nc.vector.tensor_tensor(out=lnG[:], in0=lnG[:], in1=mcorr_sb[:],
                                    op=ALU.mult)
            nc.vector.tensor_reduce(out=gcol[:], in_=lnG[:], axis=AX.X,
                                    op=ALU.add)
            lnmx = small.tile([BLOC, 1], FP)
            nc.scalar.activation(out=lnmx[:], in_=mx[:], func=AF.Ln)
            nc.vector.tensor_scalar_mul(expcol[:, 1:2], lnmx[:], -1.0)

            # last two chunks after the window-path ACT ops
            for c in range(NCH - 2, NCH):
                chunk_act(c)

            # ---------- final partition reduce ----------
            negones = small.tile([NP, 1], FP)
            nc.vector.memset(negones[:], -1.0)
            ones24 = small.tile([GROWS, 1], FP)
            nc.vector.memset(ones24[:], 1.0)
            tot_ps = psum.tile([1, NCH + 3], FP)
            nc.tensor.matmul(out=tot_ps[:, 0:NCH], lhsT=negones[:], rhs=C[:],
                             start=True, stop=True)
            nc.tensor.matmul(out=tot_ps[:, NCH:NCH + 1],
                             lhsT=ones24[:], rhs=gcol[:],
                             start=True, stop=True)
            nc.tensor.matmul(out=tot_ps[:, NCH + 1:NCH + 3],
                             lhsT=ones24[0:BLOC, :], rhs=expcol[:],
                             start=True, stop=True)
            tot = small.tile([1, 1], FP)
            nc.vector.tensor_reduce(out=tot[:], in_=tot_ps[:], axis=AX.X,
                                    op=ALU.add)
            nc.sync.dma_start(out=outd, in_=tot[:])

    nc.compile()
    _NC_CACHE = nc
    return nc


def _make_in_maps(X, lengths, tgt, w_end):
    X = np.ascontiguousarray(np.asarray(X, dtype=np.float32))
    lengths = np.asarray(lengths, dtype=np.int64)
    tgt = np.asarray(tgt, dtype=np.int64)
    w_end = np.asarray(w_end, dtype=np.int64)

    tau_s = np.maximum(0, w_end + OFFSET_D - WIN)
    tau_e = np.minimum(tau_s + WIN, lengths)
    Lw = tau_e - tau_s

    Mmat = _conv_matrix()
    I8 = np.eye(BLOC, dtype=np.float32)

    in_maps = []
    for cr in range(NCORES):
        bs = slice(cr * BLOC, (cr + 1) * BLOC)
        ls, ts, lw, tg = lengths[bs], tau_s[bs], Lw[bs], tgt[bs]

        oh = np.zeros((BLOC, K), np.float32)
        oh[np.arange(BLOC), tg] = 1.0
        ohrep = np.broadcast_to(oh[:, None, :], (BLOC, WIN, K)) \
            .reshape(BLOC, WIN * K)
        valid8 = (np.arange(WIN)[None, :] < lw[:, None]).astype(np.float32)
        auxA = np.concatenate([ohrep, valid8, I8], axis=1)  # (8, WA)

        # per-partition-row valid-element counts: row p of batch b covers
        # flat [p*FROW, (p+1)*FROW) within the batch -> thr elems valid
        r = np.arange(NP)
        thr = np.clip(ls[r // RPB] * K - (r % RPB) * FROW, 0, FROW)  # (128,)
        # scl[p, c] = -1 if chunk c of row p is fully valid else 0
        cidx = np.arange(NCH)
        scl_arr = np.where(thr[:, None] >= (cidx[None, :] + 1) * S,
                           np.float32(-1.0), np.float32(0.0))

        # boundary corrections: per batch at most one row with 0<thr<FROW;
        # its partial chunk [c0*S, thr) is re-gathered 1600-aligned.
        coff_arr = np.zeros((GROWS, 1), np.int32)
        mcorr_arr = np.zeros((GROWS, GF), np.float32)
        for b in range(BLOC):
            rows = np.where((thr > 0) & (thr < FROW)
                            & (r // RPB == b))[0]
            if len(rows) == 0:
                continue
            p0 = int(rows[0])
            th = int(thr[p0])
            c0 = th // S
            L = th - c0 * S
            if L == 0:
                continue
            g = p0 * FROW + c0 * S          # global start elem of partial
            a = (g // GF) * GF              # aligned gather start
            for u in range(GR):
                coff_arr[GR * b + u, 0] = a // GF + u
                e = a + GF * u + np.arange(GF)      # global elem idx
                mcorr_arr[GR * b + u] = np.where(
                    (e >= g) & (e < g + L), np.float32(-1.0),
                    np.float32(0.0))

        gofs_arr = (np.arange(BLOC) * T + ts).astype(np.int32) \
            .reshape(BLOC, 1)

        Xp = np.zeros(PADDED, np.float32)
        Xp[:NELEM] = X[bs].ravel()
        # chunk-major permutation: chunk c of the (NP, FROW) slab becomes
        # the contiguous block [c*NP*S, (c+1)*NP*S)
        Xcm = np.ascontiguousarray(
            Xp[:NELEM].reshape(NP, NCH, S).transpose(1, 0, 2)).ravel()
        in_maps.append({
            "Xs": Xcm,
            "Xg": Xp,
            "gofs": gofs_arr,
            "coff": coff_arr,
            "auxA": np.ascontiguousarray(auxA),
            "auxM": np.ascontiguousarray(Mmat),
            "mcorr": mcorr_arr,
            "scl": np.ascontiguousarray(scl_arr),
        })
    return in_maps


def kernel(X, lengths, tgt, w_end):
    nc = _build_program()
    in_maps = _make_in_maps(X, lengths, tgt, w_end)
    res = bass_utils.run_bass_kernel_spmd(
        nc, in_maps, core_ids=list(range(NCORES)))
    total = np.float32(0.0)
    for c in range(NCORES):
        total += np.float32(res.results[c]["out"][0, 0])
    return np.array(total, dtype=np.float32)


# revision 24
# speedup vs baseline: 1.2157x; 1.0871x over previous
"""Trainium2 Bass kernel for nn_DecoderSmoothedMaxPoolingLoss.

Loss (see reference):
  neg  = -log(1 - X)                                    (B,T,K)
  loss = sum_{b, t<len_b, k} neg
         - sum_{b, i in [0,Lw_b), k=tgt_b} neg[b, tau_s_b + i, k]
         + sum_b -log( max_j  clip(conv_same(win_b * valid_b, filt), EPS, 1) * valid_b )
  where tau_s = max(0, w_end + 40 - 60), tau_e = min(tau_s + 60, len),
  Lw = tau_e - tau_s, win_b[i] = X[b, tau_s_b + i, tgt_b].

Sharding: pure data parallel over batch - 8 batches per core on 8 cores.
Each core computes its partial scalar loss on device; host sums the 8
partials (the "all-reduce").

Per core (8 local batches = 12.8 MB), v2 layout:
  slab:     local X viewed flat as (128, 25000): partition p holds flat
            [p*25000, (p+1)*25000). 400000 elems/batch = exactly 16
            partition rows per batch, so the t<len mask is per-partition
            except for ONE partial row per batch. Loaded as NCH chunks
            of (128, S) - 12.5KB+ descriptors per partition - split
            across the HWDGE (sync) and SWDGE (gpsimd) queues.
  big term: ONE ACT instruction per chunk:
            activation(Ln, bias=1, scale=scl[:,c], accum_out=C[:,c])
            computes ln(1 - x) AND the per-partition row sum. scl is a
            host-built per-(row,chunk) mask in {-1, 0}: rows/chunks with
            any invalid element get scale 0 (ln(1) = 0 contribution).
  boundary: per batch, the one chunk that straddles t=len_b is dropped
            by scl and its valid prefix (< S elems) is re-gathered via
            indirect DMA into G (24, 1600) (3 rows of 1600 per batch,
            1600-aligned), Ln'd, and folded with a host mask in {-1,0}
            via ONE fused tensor_tensor_reduce.
  windows:  indirect DMA gathers 60 contiguous t-rows per batch into
            Wp (8, 6000); one-hot select of k = tgt via host-built
            ohrep -> win_raw (8,60).  Exclusion = sum valid*ln(1-win)
            (fused TTR).  Positive term: conv as two small matmuls,
            clip, mask, row-max, Ln.
  final:    partition sums via matmuls with (-1/+1) vectors into one
            PSUM row, free reduce -> scalar out.
"""

import numpy as np

import concourse.bass as bass
import concourse.tile as tile
from concourse import bacc
from concourse import mybir
from concourse import bass_utils
from concourse.bass import IndirectOffsetOnAxis

AF = mybir.ActivationFunctionType
ALU = mybir.AluOpType
AX = mybir.AxisListType
FP = mybir.dt.float32
I32 = mybir.dt.int32

B, T, K = 64, 4000, 100
WIN, OFFSET_D, TRUNC, SIGMA = 60, 40, 21, 9
EPS = 1e-8
NCORES = 8
BLOC = B // NCORES          # 8 batches per core
NP = 128                    # slab partitions
FROW = 25000                # elems per partition row (= T*K/16)
RPB = 16                    # partition rows per batch
NCH = 8                     # free-axis ACT sub-chunks (scl/C granularity)
S = FROW // NCH             # 3125 elems per sub-chunk per partition
NCHD = 4                    # DMA chunks (25KB descriptors)
SD = FROW // NCHD           # 6250 elems per DMA chunk per partition
NELEM = NP * FROW           # 3.2M valid elems per core
GF = 1600                   # correction gather row width
GR = 3                      # correction rows per batch
GROWS = GR * BLOC           # 24
PADDED = (NELEM // GF + 1) * GF  # 3201600: covers worst-case gather

WA = WIN * K + WIN + BLOC   # auxA (8, 6068): ohrep | valid8 | I8


def _filt_np():
    half = TRUNC // 2
    x = np.arange(-half, half + 1, dtype=np.float32)
    g = np.exp(-0.5 * (x / SIGMA) ** 2).astype(np.float32)
    g = g / g.sum()
    f = np.zeros(WIN, np.float32)
    c = WIN // 2
    f[c - half:c + half + 1] = g
    return f


def _conv_matrix():
    # smoothed[j] = sum_i win[i] * filt[i - j + pl], pl = (WIN-1)//2
    f = _filt_np()
    pl = (WIN - 1) // 2
    idx = np.arange(WIN)
    u = idx[:, None] - idx[None, :] + pl          # (i, j)
    M = np.where((u >= 0) & (u < WIN), f[np.clip(u, 0, WIN - 1)], 0.0)
    return M.astype(np.float32)


_NC_CACHE = None


def _build_program():
    global _NC_CACHE
    if _NC_CACHE is not None:
        return _NC_CACHE

    nc = bacc.Bacc("TRN2", debug=False)
    # Xs: chunk-major permuted slab (each chunk a contiguous NP*S block);
    # Xg: original layout (padded) for the indirect gathers.
    Xs = nc.dram_tensor("Xs", [NELEM], FP, kind="ExternalInput").ap()
    Xg = nc.dram_tensor("Xg", [PADDED], FP, kind="ExternalInput").ap()
    gofs = nc.dram_tensor("gofs", [BLOC, 1], I32, kind="ExternalInput").ap()
    coff = nc.dram_tensor("coff", [GROWS, 1], I32, kind="ExternalInput").ap()
    auxA = nc.dram_tensor("auxA", [BLOC, WA], FP, kind="ExternalInput").ap()
    auxM = nc.dram_tensor("auxM", [WIN, WIN], FP, kind="ExternalInput").ap()
    mcorr = nc.dram_tensor("mcorr", [GROWS, GF], FP, kind="ExternalInput").ap()
    scl = nc.dram_tensor("scl", [NP, NCH], FP, kind="ExternalInput").ap()
    outd = nc.dram_tensor("out", [1, 1], FP, kind="ExternalOutput").ap()

    with tile.TileContext(nc) as tc:
        with tc.tile_pool(name="xin", bufs=NCHD) as xin_pool, \
             tc.tile_pool(name="small", bufs=1) as small, \
             tc.tile_pool(name="psum", bufs=1, space="PSUM") as psum:

            # ---------- big slab chunks, alternating DMA queues ----------
            # DMA chunk c is the contiguous block [c*NP*SD, (c+1)*NP*SD).
            CH = NP * SD

            def chunk_dma(eng, xb, c):
                eng.dma_start(
                    out=xb[:],
                    in_=Xs[c * CH:(c + 1) * CH].rearrange(
                        "(p f) -> p f", p=NP))

            xtiles = [xin_pool.tile([NP, SD], FP, tag="xb", name=f"xb{c}")
                      for c in range(NCHD)]
            # gpsimd (SWDGE) queue: ALL small loads first (tiny, and the
            # gathers' offset inputs stay same-queue so no cross-engine
            # wait), two big chunks, then the indirect gathers (Q7 stalls
            # on the offset data, but the ring already holds 3.2MB of
            # chunk descriptors), then remaining chunks.
            gofs_sb = small.tile([BLOC, 1], I32)
            nc.gpsimd.dma_start(out=gofs_sb[:], in_=gofs)
            coff_sb = small.tile([GROWS, 1], I32)
            nc.gpsimd.dma_start(out=coff_sb[:], in_=coff)
            scl_sb = small.tile([NP, NCH], FP)
            nc.gpsimd.dma_start(out=scl_sb[:], in_=scl)
            auxA_sb = small.tile([BLOC, WA], FP)
            nc.gpsimd.dma_start(out=auxA_sb[:], in_=auxA)
            auxM_sb = small.tile([WIN, WIN], FP)
            nc.gpsimd.dma_start(out=auxM_sb[:], in_=auxM)
            mcorr_sb = small.tile([GROWS, GF], FP)
            nc.gpsimd.dma_start(out=mcorr_sb[:], in_=mcorr)
            Wp = small.tile([BLOC, WIN * K], FP)
            nc.gpsimd.indirect_dma_start(
                out=Wp[:],
                out_offset=None,
                in_=Xg[0:NELEM].rearrange("(r k) -> r k", k=K),
                in_offset=IndirectOffsetOnAxis(ap=gofs_sb[:, :1], axis=0),
            )
            G = small.tile([GROWS, GF], FP)
            nc.gpsimd.indirect_dma_start(
                out=G[:],
                out_offset=None,
                in_=Xg.rearrange("(r f) -> r f", f=GF),
                in_offset=IndirectOffsetOnAxis(ap=coff_sb[:, :1], axis=0),
            )

            # sync (HWDGE) queue: even big chunks; scalar (HWDGE) queue:
            # odd big chunks (dispatched from the head of the ACT stream,
            # no deps so they never stall it)
            chunk_dma(nc.sync, xtiles[0], 0)
            chunk_dma(nc.scalar, xtiles[1], 1)
            chunk_dma(nc.sync, xtiles[2], 2)
            chunk_dma(nc.scalar, xtiles[3], 3)

            ohrep_sl = auxA_sb[0:BLOC, 0:WIN * K]
            valid_sl = auxA_sb[0:BLOC, WIN * K:WIN * K + WIN]
            I8_sl = auxA_sb[0:BLOC, WIN * K + WIN:WA]

            C = small.tile([NP, NCH], FP)
            nc.vector.memset(C[:], 0.0)

            # ---------- correction path Ln first (data arrives early);
            # its DVE ops are emitted after the window chain ----------
            lnG = small.tile([GROWS, GF], FP)
            nc.scalar.activation(out=lnG[:], in_=G[:], func=AF.Ln,
                                 bias=1.0, scale=-1.0)

            # ---------- big term: ONE fused ACT op per sub-chunk ----------
            def chunk_act(c):
                xb = xtiles[c // 2]
                h = (c % 2) * S
                nc.scalar.activation(out=xb[:, h:h + S],
                                     in_=xb[:, h:h + S], func=AF.Ln,
                                     bias=1.0, scale=scl_sb[:, c:c + 1],
                                     accum_out=C[:, c:c + 1])

            for c in range(NCH - 2):
                chunk_act(c)

            # ---------- window path (ACT ops land before last chunks) ----
            nc.vector.tensor_tensor(out=Wp[:], in0=Wp[:], in1=ohrep_sl,
                                    op=ALU.mult)
            win_raw = small.tile([BLOC, WIN], FP)
            nc.vector.tensor_reduce(
                out=win_raw[:],
                in_=Wp[:].rearrange("b (i k) -> b i k", k=K),
                axis=AX.X, op=ALU.add)
            # positive term part 1: win_v = win_raw * valid
            winv = small.tile([BLOC, WIN], FP)
            nc.vector.tensor_tensor(out=winv[:], in0=win_raw[:],
                                    in1=valid_sl, op=ALU.mult)
            # exclusion: + sum_i valid * ln(1 - win_raw), fused
            expcol = small.tile([BLOC, 2], FP)
            lnw = small.tile([BLOC, WIN], FP)
            nc.scalar.activation(out=lnw[:], in_=win_raw[:], func=AF.Ln,
                                 bias=1.0, scale=-1.0)
            lnwv = small.tile([BLOC, WIN], FP)
            nc.vector.tensor_tensor(out=lnwv[:], in0=lnw[:], in1=valid_sl,
                                    op=ALU.mult)
            nc.vector.tensor_reduce(out=expcol[:, 0:1], in_=lnwv[:],
                                    axis=AX.X, op=ALU.add)
            # smoothed = win_v @ M (transpose first via identity)
            wvt_ps = psum.tile([WIN, BLOC], FP)
            nc.tensor.matmul(out=wvt_ps[:], lhsT=winv[:], rhs=I8_sl,
                             start=True, stop=True)
            wvt = small.tile([WIN, BLOC], FP)
            nc.vector.tensor_copy(out=wvt[:], in_=wvt_ps[:])
            sm_ps = psum.tile([BLOC, WIN], FP)
            nc.tensor.matmul(out=sm_ps[:], lhsT=wvt[:], rhs=auxM_sb[:],
                             start=True, stop=True)
            smc = small.tile([BLOC, WIN], FP)
            nc.vector.tensor_scalar(out=smc[:], in0=sm_ps[:],
                                    scalar1=EPS, scalar2=1.0,
                                    op0=ALU.max, op1=ALU.min)
            smv = small.tile([BLOC, WIN], FP)
            nc.vector.tensor_tensor(out=smv[:], in0=smc[:], in1=valid_sl,
                                    op=ALU.mult)
            mx = small.tile([BLOC, 1], FP)
            nc.vector.tensor_reduce(out=mx[:], in_=smv[:], axis=AX.X,
                                    op=ALU.max)
            # correction fold (DVE) after the window chain
            gcol = small.tile([GROWS, 1], FP)
            nc.vector.tensor_tensor(out=lnG[:], in0=lnG[:], in1=mcorr_sb[:],
                                    op=ALU.mult)
            nc.vector.tensor_reduce(out=gcol[:], in_=lnG[:], axis=AX.X,
                                    op=ALU.add)
            lnmx = small.tile([BLOC, 1], FP)
            nc.scalar.activation(out=lnmx[:], in_=mx[:], func=AF.Ln)
            nc.vector.tensor_scalar_mul(expcol[:, 1:2], lnmx[:], -1.0)

            # last two chunks after the window-path ACT ops
            for c in range(NCH - 2, NCH):
                chunk_act(c)

            # ---------- final partition reduce ----------
            negones = small.tile([NP, 1], FP)
            nc.vector.memset(negones[:], -1.0)
            ones24 = small.tile([GROWS, 1], FP)
            nc.vector.memset(ones24[:], 1.0)
            tot_ps = psum.tile([1, NCH + 3], FP)
            nc.tensor.matmul(out=tot_ps[:, 0:NCH], lhsT=negones[:], rhs=C[:],
                             start=True, stop=True)
            nc.tensor.matmul(out=tot_ps[:, NCH:NCH + 1],
                             lhsT=ones24[:], rhs=gcol[:],
                             start=True, stop=True)
            nc.tensor.matmul(out=tot_ps[:, NCH + 1:NCH + 3],
                             lhsT=ones24[0:BLOC, :], rhs=expcol[:],
                             start=True, stop=True)
            tot = small.tile([1, 1], FP)
            nc.vector.tensor_reduce(out=tot[:], in_=tot_ps[:], axis=AX.X,
                                    op=ALU.add)
            nc.sync.dma_start(out=outd, in_=tot[:])

    nc.compile()
    _NC_CACHE = nc
    return nc


def _make_in_maps(X, lengths, tgt, w_end):
    X = np.ascontiguousarray(np.asarray(X, dtype=np.float32))
    lengths = np.asarray(lengths, dtype=np.int64)
    tgt = np.asarray(tgt, dtype=np.int64)
    w_end = np.asarray(w_end, dtype=np.int64)

    tau_s = np.maximum(0, w_end + OFFSET_D - WIN)
    tau_e = np.minimum(tau_s + WIN, lengths)
    Lw = tau_e - tau_s

    Mmat = _conv_matrix()
    I8 = np.eye(BLOC, dtype=np.float32)

    in_maps = []
    for cr in range(NCORES):
        bs = slice(cr * BLOC, (cr + 1) * BLOC)
        ls, ts, lw, tg = lengths[bs], tau_s[bs], Lw[bs], tgt[bs]

        oh = np.zeros((BLOC, K), np.float32)
        oh[np.arange(BLOC), tg] = 1.0
        ohrep = np.broadcast_to(oh[:, None, :], (BLOC, WIN, K)) \
            .reshape(BLOC, WIN * K)
        valid8 = (np.arange(WIN)[None, :] < lw[:, None]).astype(np.float32)
        auxA = np.concatenate([ohrep, valid8, I8], axis=1)  # (8, WA)

        # per-partition-row valid-element counts: row p of batch b covers
        # flat [p*FROW, (p+1)*FROW) within the batch -> thr elems valid
        r = np.arange(NP)
        thr = np.clip(ls[r // RPB] * K - (r % RPB) * FROW, 0, FROW)  # (128,)
        # scl[p, c] = -1 if chunk c of row p is fully valid else 0
        cidx = np.arange(NCH)
        scl_arr = np.where(thr[:, None] >= (cidx[None, :] + 1) * S,
                           np.float32(-1.0), np.float32(0.0))

        # boundary corrections: per batch at most one row with 0<thr<FROW;
        # its partial chunk [c0*S, thr) is re-gathered 1600-aligned.
        coff_arr = np.zeros((GROWS, 1), np.int32)
        mcorr_arr = np.zeros((GROWS, GF), np.float32)
        for b in range(BLOC):
            rows = np.where((thr > 0) & (thr < FROW)
                            & (r // RPB == b))[0]
            if len(rows) == 0:
                continue
            p0 = int(rows[0])
            th = int(thr[p0])
            c0 = th // S
            L = th - c0 * S
            if L == 0:
                continue
            g = p0 * FROW + c0 * S          # global start elem of partial
            a = (g // GF) * GF              # aligned gather start
            for u in range(GR):
                coff_arr[GR * b + u, 0] = a // GF + u
                e = a + GF * u + np.arange(GF)      # global elem idx
                mcorr_arr[GR * b + u] = np.where(
                    (e >= g) & (e < g + L), np.float32(-1.0),
                    np.float32(0.0))

        gofs_arr = (np.arange(BLOC) * T + ts).astype(np.int32) \
            .reshape(BLOC, 1)

        Xp = np.zeros(PADDED, np.float32)
        Xp[:NELEM] = X[bs].ravel()
        # chunk-major permutation: DMA chunk c of the (NP, FROW) slab
        # becomes the contiguous block [c*NP*SD, (c+1)*NP*SD)
        Xcm = np.ascontiguousarray(
            Xp[:NELEM].reshape(NP, NCHD, SD).transpose(1, 0, 2)).ravel()
        in_maps.append({
            "Xs": Xcm,
            "Xg": Xp,
            "gofs": gofs_arr,
            "coff": coff_arr,
            "auxA": np.ascontiguousarray(auxA),
            "auxM": np.ascontiguousarray(Mmat),
            "mcorr": mcorr_arr,
            "scl": np.ascontiguousarray(scl_arr),
        })
    return in_maps


def kernel(X, lengths, tgt, w_end):
    nc = _build_program()
    in_maps = _make_in_maps(X, lengths, tgt, w_end)
    res = bass_utils.run_bass_kernel_spmd(
        nc, in_maps, core_ids=list(range(NCORES)))
    total = np.float32(0.0)
    for c in range(NCORES):
        total += np.float32(res.results[c]["out"][0, 0])
    return np.array(total, dtype=np.float32)


# revision 33
# speedup vs baseline: 1.2436x; 1.0230x over previous
"""Trainium2 Bass kernel for nn_DecoderSmoothedMaxPoolingLoss.

Loss (see reference):
  neg  = -log(1 - X)                                    (B,T,K)
  loss = sum_{b, t<len_b, k} neg
         - sum_{b, i in [0,Lw_b), k=tgt_b} neg[b, tau_s_b + i, k]
         + sum_b -log( max_j  clip(conv_same(win_b * valid_b, filt), EPS, 1) * valid_b )
  where tau_s = max(0, w_end + 40 - 60), tau_e = min(tau_s + 60, len),
  Lw = tau_e - tau_s, win_b[i] = X[b, tau_s_b + i, tgt_b].

Sharding: pure data parallel over batch - 8 batches per core on 8 cores.
Each core computes its partial scalar loss on device; host sums the 8
partials (the "all-reduce").

Per core (8 local batches = 12.8 MB), v2 layout:
  slab:     local X viewed flat as (128, 25000): partition p holds flat
            [p*25000, (p+1)*25000). 400000 elems/batch = exactly 16
            partition rows per batch, so the t<len mask is per-partition
            except for ONE partial row per batch. Loaded as NCH chunks
            of (128, S) - 12.5KB+ descriptors per partition - split
            across the HWDGE (sync) and SWDGE (gpsimd) queues.
  big term: ONE ACT instruction per chunk:
            activation(Ln, bias=1, scale=scl[:,c], accum_out=C[:,c])
            computes ln(1 - x) AND the per-partition row sum. scl is a
            host-built per-(row,chunk) mask in {-1, 0}: rows/chunks with
            any invalid element get scale 0 (ln(1) = 0 contribution).
  boundary: per batch, the one chunk that straddles t=len_b is dropped
            by scl and its valid prefix (< S elems) is re-gathered via
            indirect DMA into G (24, 1600) (3 rows of 1600 per batch,
            1600-aligned), Ln'd, and folded with a host mask in {-1,0}
            via ONE fused tensor_tensor_reduce.
  windows:  indirect DMA gathers 60 contiguous t-rows per batch into
            Wp (8, 6000); one-hot select of k = tgt via host-built
            ohrep -> win_raw (8,60).  Exclusion = sum valid*ln(1-win)
            (fused TTR).  Positive term: conv as two small matmuls,
            clip, mask, row-max, Ln.
  final:    partition sums via matmuls with (-1/+1) vectors into one
            PSUM row, free reduce -> scalar out.
"""

import numpy as np

import concourse.bass as bass
import concourse.tile as tile
from concourse import bacc
from concourse import mybir
from concourse import bass_utils
from concourse.bass import IndirectOffsetOnAxis

AF = mybir.ActivationFunctionType
ALU = mybir.AluOpType
AX = mybir.AxisListType
FP = mybir.dt.float32
I32 = mybir.dt.int32

B, T, K = 64, 4000, 100
WIN, OFFSET_D, TRUNC, SIGMA = 60, 40, 21, 9
EPS = 1e-8
NCORES = 8
BLOC = B // NCORES          # 8 batches per core
NP = 128                    # slab partitions
FROW = 25000                # elems per partition row (= T*K/16)
RPB = 16                    # partition rows per batch
NCH = 8                     # free-axis ACT sub-chunks (scl/C granularity)
S = FROW // NCH             # 3125 elems per sub-chunk per partition
NCHD = 4                    # DMA chunks (25KB descriptors)
SD = FROW // NCHD           # 6250 elems per DMA chunk per partition
NELEM = NP * FROW           # 3.2M valid elems per core
GF = 1600                   # correction gather row width
GR = 3                      # correction rows per batch
GROWS = GR * BLOC           # 24
PADDED = (NELEM // GF + 1) * GF  # 3201600: covers worst-case gather

WA = WIN * K + WIN + BLOC   # auxA (8, 6068): ohrep | valid8 | I8


def _filt_np():
    half = TRUNC // 2
    x = np.arange(-half, half + 1, dtype=np.float32)
    g = np.exp(-0.5 * (x / SIGMA) ** 2).astype(np.float32)
    g = g / g.sum()
    f = np.zeros(WIN, np.float32)
    c = WIN // 2
    f[c - half:c + half + 1] = g
    return f


def _conv_matrix():
    # smoothed[j] = sum_i win[i] * filt[i - j + pl], pl = (WIN-1)//2
    f = _filt_np()
    pl = (WIN - 1) // 2
    idx = np.arange(WIN)
    u = idx[:, None] - idx[None, :] + pl          # (i, j)
    M = np.where((u >= 0) & (u < WIN), f[np.clip(u, 0, WIN - 1)], 0.0)
    return M.astype(np.float32)


_NC_CACHE = None


def _build_program():
    global _NC_CACHE
    if _NC_CACHE is not None:
        return _NC_CACHE

    nc = bacc.Bacc("TRN2", debug=False)
    # Xs: chunk-major permuted slab (each chunk a contiguous NP*S block);
    # Xg: original layout (padded) for the indirect gathers.
    Xs = nc.dram_tensor("Xs", [NELEM], FP, kind="ExternalInput").ap()
    Xg = nc.dram_tensor("Xg", [PADDED], FP, kind="ExternalInput").ap()
    # offs col 0 = correction row offsets (24), col 1 rows 0-7 = window
    # row offsets; column layout keeps both gather offset APs at
    # partition offset 0.
    offs = nc.dram_tensor("offs", [GROWS, 2], I32, kind="ExternalInput").ap()
    auxA = nc.dram_tensor("auxA", [BLOC, WA], FP, kind="ExternalInput").ap()
    auxM = nc.dram_tensor("auxM", [WIN, WIN], FP, kind="ExternalInput").ap()
    mcorr = nc.dram_tensor("mcorr", [GROWS, GF], FP, kind="ExternalInput").ap()
    scl = nc.dram_tensor("scl", [NP, NCH], FP, kind="ExternalInput").ap()
    outd = nc.dram_tensor("out", [1, 1], FP, kind="ExternalOutput").ap()

    with tile.TileContext(nc) as tc:
        with tc.tile_pool(name="xin", bufs=NCHD) as xin_pool, \
             tc.tile_pool(name="small", bufs=1) as small, \
             tc.tile_pool(name="psum", bufs=1, space="PSUM") as psum:

            # ---------- big slab chunks, alternating DMA queues ----------
            # DMA chunk c is the contiguous block [c*NP*SD, (c+1)*NP*SD).
            CH = NP * SD

            def chunk_dma(eng, xb, c):
                eng.dma_start(
                    out=xb[:],
                    in_=Xs[c * CH:(c + 1) * CH].rearrange(
                        "(p f) -> p f", p=NP))

            xtiles = [xin_pool.tile([NP, SD], FP, tag="xb", name=f"xb{c}")
                      for c in range(NCHD)]
            # gpsimd (SWDGE) queue carries ONLY the latency-critical
            # loads: offsets then the two indirect gathers. SWDGE
            # serializes dma_starts (~2-3us dead time between each), so
            # nothing else may queue here.
            offs_sb = small.tile([GROWS, 2], I32)
            nc.gpsimd.dma_start(out=offs_sb[:], in_=offs)
            Wp = small.tile([BLOC, WIN * K], FP)
            nc.gpsimd.indirect_dma_start(
                out=Wp[:],
                out_offset=None,
                in_=Xg[0:NELEM].rearrange("(r k) -> r k", k=K),
                in_offset=IndirectOffsetOnAxis(
                    ap=offs_sb[0:BLOC, 1:2], axis=0),
            )
            G = small.tile([GROWS, GF], FP)
            nc.gpsimd.indirect_dma_start(
                out=G[:],
                out_offset=None,
                in_=Xg.rearrange("(r f) -> r f", f=GF),
                in_offset=IndirectOffsetOnAxis(
                    ap=offs_sb[0:GROWS, 0:1], axis=0),
            )

            # sync (HWDGE) queue: chunk0, the mid-kernel aux loads, chunk2
            chunk_dma(nc.sync, xtiles[0], 0)
            auxA_sb = small.tile([BLOC, WA], FP)
            nc.sync.dma_start(out=auxA_sb[:], in_=auxA)
            auxM_sb = small.tile([WIN, WIN], FP)
            nc.sync.dma_start(out=auxM_sb[:], in_=auxM)
            mcorr_sb = small.tile([GROWS, GF], FP)
            nc.sync.dma_start(out=mcorr_sb[:], in_=mcorr)
            chunk_dma(nc.sync, xtiles[2], 2)

            # scalar (HWDGE) queue, dispatched from the head of the ACT
            # stream (no deps so they never stall it): scl + odd chunks
            scl_sb = small.tile([NP, NCH], FP)
            nc.scalar.dma_start(out=scl_sb[:], in_=scl)
            chunk_dma(nc.scalar, xtiles[1], 1)
            chunk_dma(nc.scalar, xtiles[3], 3)

            ohrep_sl = auxA_sb[0:BLOC, 0:WIN * K]
            valid_sl = auxA_sb[0:BLOC, WIN * K:WIN * K + WIN]
            I8_sl = auxA_sb[0:BLOC, WIN * K + WIN:WA]

            C = small.tile([NP, NCH], FP)
            nc.vector.memset(C[:], 0.0)

            # ---------- correction path Ln first (data arrives early);
            # its DVE ops are emitted after the window chain ----------
            lnG = small.tile([GROWS, GF], FP)
            nc.scalar.activation(out=lnG[:], in_=G[:], func=AF.Ln,
                                 bias=1.0, scale=-1.0)

            # ---------- big term: ONE fused ACT op per sub-chunk ----------
            def chunk_act(c):
                xb = xtiles[c // 2]
                h = (c % 2) * S
                nc.scalar.activation(out=xb[:, h:h + S],
                                     in_=xb[:, h:h + S], func=AF.Ln,
                                     bias=1.0, scale=scl_sb[:, c:c + 1],
                                     accum_out=C[:, c:c + 1])

            for c in range(NCH - 2):
                chunk_act(c)

            # ---------- window path (ACT ops land before last chunks) ----
            nc.vector.tensor_tensor(out=Wp[:], in0=Wp[:], in1=ohrep_sl,
                                    op=ALU.mult)
            win_raw = small.tile([BLOC, WIN], FP)
            nc.vector.tensor_reduce(
                out=win_raw[:],
                in_=Wp[:].rearrange("b (i k) -> b i k", k=K),
                axis=AX.X, op=ALU.add)
            # positive term part 1: win_v = win_raw * valid
            winv = small.tile([BLOC, WIN], FP)
            nc.vector.tensor_tensor(out=winv[:], in0=win_raw[:],
                                    in1=valid_sl, op=ALU.mult)
            # exclusion: + sum_i valid * ln(1 - win_raw), fused
            expcol = small.tile([BLOC, 2], FP)
            lnw = small.tile([BLOC, WIN], FP)
            nc.scalar.activation(out=lnw[:], in_=win_raw[:], func=AF.Ln,
                                 bias=1.0, scale=-1.0)
            lnwv = small.tile([BLOC, WIN], FP)
            nc.vector.tensor_tensor(out=lnwv[:], in0=lnw[:], in1=valid_sl,
                                    op=ALU.mult)
            nc.vector.tensor_reduce(out=expcol[:, 0:1], in_=lnwv[:],
                                    axis=AX.X, op=ALU.add)
            # smoothed = win_v @ M (transpose first via identity)
            wvt_ps = psum.tile([WIN, BLOC], FP)
            nc.tensor.matmul(out=wvt_ps[:], lhsT=winv[:], rhs=I8_sl,
                             start=True, stop=True)
            wvt = small.tile([WIN, BLOC], FP)
            nc.vector.tensor_copy(out=wvt[:], in_=wvt_ps[:])
            sm_ps = psum.tile([BLOC, WIN], FP)
            nc.tensor.matmul(out=sm_ps[:], lhsT=wvt[:], rhs=auxM_sb[:],
                             start=True, stop=True)
            smc = small.tile([BLOC, WIN], FP)
            nc.vector.tensor_scalar(out=smc[:], in0=sm_ps[:],
                                    scalar1=EPS, scalar2=1.0,
                                    op0=ALU.max, op1=ALU.min)
            smv = small.tile([BLOC, WIN], FP)
            nc.vector.tensor_tensor(out=smv[:], in0=smc[:], in1=valid_sl,
                                    op=ALU.mult)
            mx = small.tile([BLOC, 1], FP)
            nc.vector.tensor_reduce(out=mx[:], in_=smv[:], axis=AX.X,
                                    op=ALU.max)
            # correction fold (DVE) after the window chain
            gcol = small.tile([GROWS, 1], FP)
            nc.vector.tensor_tensor(out=lnG[:], in0=lnG[:], in1=mcorr_sb[:],
                                    op=ALU.mult)
            nc.vector.tensor_reduce(out=gcol[:], in_=lnG[:], axis=AX.X,
                                    op=ALU.add)
            lnmx = small.tile([BLOC, 1], FP)
            nc.scalar.activation(out=lnmx[:], in_=mx[:], func=AF.Ln)
            nc.vector.tensor_scalar_mul(expcol[:, 1:2], lnmx[:], -1.0)

            # last two chunks after the window-path ACT ops
            for c in range(NCH - 2, NCH):
                chunk_act(c)

            # ---------- final partition reduce ----------
            negones = small.tile([NP, 1], FP)
            nc.vector.memset(negones[:], -1.0)
            ones24 = small.tile([GROWS, 1], FP)
            nc.vector.memset(ones24[:], 1.0)
            tot_ps = psum.tile([1, NCH + 3], FP)
            nc.tensor.matmul(out=tot_ps[:, 0:NCH], lhsT=negones[:], rhs=C[:],
                             start=True, stop=True)
            nc.tensor.matmul(out=tot_ps[:, NCH:NCH + 1],
                             lhsT=ones24[:], rhs=gcol[:],
                             start=True, stop=True)
            nc.tensor.matmul(out=tot_ps[:, NCH + 1:NCH + 3],
                             lhsT=ones24[0:BLOC, :], rhs=expcol[:],
                             start=True, stop=True)
            tot = small.tile([1, 1], FP)
            nc.vector.tensor_reduce(out=tot[:], in_=tot_ps[:], axis=AX.X,
                                    op=ALU.add)
            nc.sync.dma_start(out=outd, in_=tot[:])

    nc.compile()
    _NC_CACHE = nc
    return nc


def _make_in_maps(X, lengths, tgt, w_end):
    X = np.ascontiguousarray(np.asarray(X, dtype=np.float32))
    lengths = np.asarray(lengths, dtype=np.int64)
    tgt = np.asarray(tgt, dtype=np.int64)
    w_end = np.asarray(w_end, dtype=np.int64)

    tau_s = np.maximum(0, w_end + OFFSET_D - WIN)
    tau_e = np.minimum(tau_s + WIN, lengths)
    Lw = tau_e - tau_s

    Mmat = _conv_matrix()
    I8 = np.eye(BLOC, dtype=np.float32)

    in_maps = []
    for cr in range(NCORES):
        bs = slice(cr * BLOC, (cr + 1) * BLOC)
        ls, ts, lw, tg = lengths[bs], tau_s[bs], Lw[bs], tgt[bs]

        oh = np.zeros((BLOC, K), np.float32)
        oh[np.arange(BLOC), tg] = 1.0
        ohrep = np.broadcast_to(oh[:, None, :], (BLOC, WIN, K)) \
            .reshape(BLOC, WIN * K)
        valid8 = (np.arange(WIN)[None, :] < lw[:, None]).astype(np.float32)
        auxA = np.concatenate([ohrep, valid8, I8], axis=1)  # (8, WA)

        # per-partition-row valid-element counts: row p of batch b covers
        # flat [p*FROW, (p+1)*FROW) within the batch -> thr elems valid
        r = np.arange(NP)
        thr = np.clip(ls[r // RPB] * K - (r % RPB) * FROW, 0, FROW)  # (128,)
        # scl[p, c] = -1 if chunk c of row p is fully valid else 0
        cidx = np.arange(NCH)
        scl_arr = np.where(thr[:, None] >= (cidx[None, :] + 1) * S,
                           np.float32(-1.0), np.float32(0.0))

        # boundary corrections: per batch at most one row with 0<thr<FROW;
        # its partial chunk [c0*S, thr) is re-gathered 1600-aligned.
        coff_arr = np.zeros((GROWS, 1), np.int32)
        mcorr_arr = np.zeros((GROWS, GF), np.float32)
        for b in range(BLOC):
            rows = np.where((thr > 0) & (thr < FROW)
                            & (r // RPB == b))[0]
            if len(rows) == 0:
                continue
            p0 = int(rows[0])
            th = int(thr[p0])
            c0 = th // S
            L = th - c0 * S
            if L == 0:
                continue
            g = p0 * FROW + c0 * S          # global start elem of partial
            a = (g // GF) * GF              # aligned gather start
            for u in range(GR):
                coff_arr[GR * b + u, 0] = a // GF + u
                e = a + GF * u + np.arange(GF)      # global elem idx
                mcorr_arr[GR * b + u] = np.where(
                    (e >= g) & (e < g + L), np.float32(-1.0),
                    np.float32(0.0))

        gofs_arr = (np.arange(BLOC) * T + ts).astype(np.int32) \
            .reshape(BLOC, 1)
        offs_arr = np.zeros((GROWS, 2), np.int32)
        offs_arr[:, 0:1] = coff_arr
        offs_arr[0:BLOC, 1:2] = gofs_arr

        Xp = np.zeros(PADDED, np.float32)
        Xp[:NELEM] = X[bs].ravel()
        # chunk-major permutation: DMA chunk c of the (NP, FROW) slab
        # becomes the contiguous block [c*NP*SD, (c+1)*NP*SD)
        Xcm = np.ascontiguousarray(
            Xp[:NELEM].reshape(NP, NCHD, SD).transpose(1, 0, 2)).ravel()
        in_maps.append({
            "Xs": Xcm,
            "Xg": Xp,
            "offs": offs_arr,
            "auxA": np.ascontiguousarray(auxA),
            "auxM": np.ascontiguousarray(Mmat),
            "mcorr": mcorr_arr,
            "scl": np.ascontiguousarray(scl_arr),
        })
    return in_maps


def kernel(X, lengths, tgt, w_end):
    nc = _build_program()
    in_maps = _make_in_maps(X, lengths, tgt, w_end)
    res = bass_utils.run_bass_kernel_spmd(
        nc, in_maps, core_ids=list(range(NCORES)))
    total = np.float32(0.0)
    for c in range(NCORES):
        total += np.float32(res.results[c]["out"][0, 0])
    return np.array(total, dtype=np.float32)


# revision 39
# speedup vs baseline: 1.3843x; 1.1132x over previous
"""Trainium2 Bass kernel for nn_DecoderSmoothedMaxPoolingLoss.

Loss (see reference):
  neg  = -log(1 - X)                                    (B,T,K)
  loss = sum_{b, t<len_b, k} neg
         - sum_{b, i in [0,Lw_b), k=tgt_b} neg[b, tau_s_b + i, k]
         + sum_b -log( max_j  clip(conv_same(win_b * valid_b, filt), EPS, 1) * valid_b )
  where tau_s = max(0, w_end + 40 - 60), tau_e = min(tau_s + 60, len),
  Lw = tau_e - tau_s, win_b[i] = X[b, tau_s_b + i, tgt_b].

Sharding: pure data parallel over batch - 8 batches per core on 8 cores.
Each core computes its partial scalar loss on device; host sums the 8
partials (the "all-reduce").

Per core (8 local batches = 12.8 MB), v2 layout:
  slab:     local X viewed flat as (128, 25000): partition p holds flat
            [p*25000, (p+1)*25000). 400000 elems/batch = exactly 16
            partition rows per batch, so the t<len mask is per-partition
            except for ONE partial row per batch. Loaded as NCH chunks
            of (128, S) - 12.5KB+ descriptors per partition - split
            across the HWDGE (sync) and SWDGE (gpsimd) queues.
  big term: ONE ACT instruction per chunk:
            activation(Ln, bias=1, scale=scl[:,c], accum_out=C[:,c])
            computes ln(1 - x) AND the per-partition row sum. scl is a
            host-built per-(row,chunk) mask in {-1, 0}: rows/chunks with
            any invalid element get scale 0 (ln(1) = 0 contribution).
  boundary: per batch, the one chunk that straddles t=len_b is dropped
            by scl and its valid prefix (< S elems) is re-gathered via
            indirect DMA into G (24, 1600) (3 rows of 1600 per batch,
            1600-aligned), Ln'd, and folded with a host mask in {-1,0}
            via ONE fused tensor_tensor_reduce.
  windows:  indirect DMA gathers 60 contiguous t-rows per batch into
            Wp (8, 6000); one-hot select of k = tgt via host-built
            ohrep -> win_raw (8,60).  Exclusion = sum valid*ln(1-win)
            (fused TTR).  Positive term: conv as two small matmuls,
            clip, mask, row-max, Ln.
  final:    partition sums via matmuls with (-1/+1) vectors into one
            PSUM row, free reduce -> scalar out.
"""

import numpy as np

import concourse.bass as bass
import concourse.tile as tile
from concourse import bacc
from concourse import mybir
from concourse import bass_utils
from concourse.bass import IndirectOffsetOnAxis

AF = mybir.ActivationFunctionType
ALU = mybir.AluOpType
AX = mybir.AxisListType
FP = mybir.dt.float32
I32 = mybir.dt.int32

B, T, K = 64, 4000, 100
WIN, OFFSET_D, TRUNC, SIGMA = 60, 40, 21, 9
EPS = 1e-8
NCORES = 8
BLOC = B // NCORES          # 8 batches per core
NP = 128                    # slab partitions
FROW = 25000                # elems per partition row (= T*K/16)
RPB = 16                    # partition rows per batch
NCH = 8                     # free-axis ACT sub-chunks (scl/C granularity)
S = FROW // NCH             # 3125 elems per sub-chunk per partition
# DMA chunk spans in S units: small first chunks (fast pipeline start),
# big middle chunks (25KB descriptors), small last chunks (short tail).
CHUNK_SPANS = [(0, 1), (1, 2), (2, 4), (4, 6), (6, 7), (7, 8)]
NELEM = NP * FROW           # 3.2M valid elems per core
GF = 1600                   # correction gather row width
GR = 3                      # correction rows per batch
GROWS = GR * BLOC           # 24
PADDED = (NELEM // GF + 1) * GF  # 3201600: covers worst-case gather

WA = WIN * K + WIN + BLOC   # auxA (8, 6068): ohrep | valid8 | I8


def _filt_np():
    half = TRUNC // 2
    x = np.arange(-half, half + 1, dtype=np.float32)
    g = np.exp(-0.5 * (x / SIGMA) ** 2).astype(np.float32)
    g = g / g.sum()
    f = np.zeros(WIN, np.float32)
    c = WIN // 2
    f[c - half:c + half + 1] = g
    return f


def _conv_matrix():
    # smoothed[j] = sum_i win[i] * filt[i - j + pl], pl = (WIN-1)//2
    f = _filt_np()
    pl = (WIN - 1) // 2
    idx = np.arange(WIN)
    u = idx[:, None] - idx[None, :] + pl          # (i, j)
    M = np.where((u >= 0) & (u < WIN), f[np.clip(u, 0, WIN - 1)], 0.0)
    return M.astype(np.float32)


_NC_CACHE = None


def _build_program():
    global _NC_CACHE
    if _NC_CACHE is not None:
        return _NC_CACHE

    nc = bacc.Bacc("TRN2", debug=False)
    # Xs: chunk-major permuted slab (each chunk a contiguous NP*S block);
    # Xg: original layout (padded) for the indirect gathers.
    Xs = nc.dram_tensor("Xs", [NELEM], FP, kind="ExternalInput").ap()
    Xg = nc.dram_tensor("Xg", [PADDED], FP, kind="ExternalInput").ap()
    # offs col 0 = correction row offsets (24), col 1 rows 0-7 = window
    # row offsets; column layout keeps both gather offset APs at
    # partition offset 0.
    offs = nc.dram_tensor("offs", [GROWS, 2], I32, kind="ExternalInput").ap()
    auxA = nc.dram_tensor("auxA", [BLOC, WA], FP, kind="ExternalInput").ap()
    auxM = nc.dram_tensor("auxM", [WIN, WIN], FP, kind="ExternalInput").ap()
    mcorr = nc.dram_tensor("mcorr", [GROWS, GF], FP, kind="ExternalInput").ap()
    scl = nc.dram_tensor("scl", [NP, NCH], FP, kind="ExternalInput").ap()
    outd = nc.dram_tensor("out", [1, 1], FP, kind="ExternalOutput").ap()

    with tile.TileContext(nc) as tc:
        with tc.tile_pool(name="xin", bufs=1) as xin_pool, \
             tc.tile_pool(name="small", bufs=1) as small, \
             tc.tile_pool(name="psum", bufs=1, space="PSUM") as psum:

            # ---------- big slab chunks, alternating DMA queues ----------
            # DMA chunk i covers sub-chunks [a, b): the contiguous block
            # [a*NP*S, b*NP*S) of chunk-major Xs.
            def chunk_dma(eng, xb, i):
                a, b = CHUNK_SPANS[i]
                eng.dma_start(
                    out=xb[:],
                    in_=Xs[a * NP * S:b * NP * S].rearrange(
                        "(p f) -> p f", p=NP))

            xtiles = [xin_pool.tile([NP, (b - a) * S], FP, tag=f"xb{i}",
                                    name=f"xb{i}")
                      for i, (a, b) in enumerate(CHUNK_SPANS)]
            # sub-chunk c -> (DMA chunk index, column offset in its tile)
            sub_loc = {}
            for i, (a, b) in enumerate(CHUNK_SPANS):
                for s in range(a, b):
                    sub_loc[s] = (i, (s - a) * S)
            # gpsimd (SWDGE) queue carries ONLY the latency-critical
            # loads: offsets then the two indirect gathers. SWDGE
            # serializes dma_starts (~2-3us dead time between each), so
            # nothing else may queue here.
            offs_sb = small.tile([GROWS, 2], I32)
            nc.gpsimd.dma_start(out=offs_sb[:], in_=offs)
            Wp = small.tile([BLOC, WIN * K], FP)
            nc.gpsimd.indirect_dma_start(
                out=Wp[:],
                out_offset=None,
                in_=Xg[0:NELEM].rearrange("(r k) -> r k", k=K),
                in_offset=IndirectOffsetOnAxis(
                    ap=offs_sb[0:BLOC, 1:2], axis=0),
            )
            G = small.tile([GROWS, GF], FP)
            nc.gpsimd.indirect_dma_start(
                out=G[:],
                out_offset=None,
                in_=Xg.rearrange("(r f) -> r f", f=GF),
                in_offset=IndirectOffsetOnAxis(
                    ap=offs_sb[0:GROWS, 0:1], axis=0),
            )

            # sync (HWDGE) queue: even chunks + the mid-kernel aux loads
            chunk_dma(nc.sync, xtiles[0], 0)
            auxA_sb = small.tile([BLOC, WA], FP)
            nc.sync.dma_start(out=auxA_sb[:], in_=auxA)
            auxM_sb = small.tile([WIN, WIN], FP)
            nc.sync.dma_start(out=auxM_sb[:], in_=auxM)
            mcorr_sb = small.tile([GROWS, GF], FP)
            nc.sync.dma_start(out=mcorr_sb[:], in_=mcorr)
            chunk_dma(nc.sync, xtiles[2], 2)
            chunk_dma(nc.sync, xtiles[4], 4)

            # scalar (HWDGE) queue, dispatched from the head of the ACT
            # stream (high_priority pins them there): scl + odd chunks
            scl_sb = small.tile([NP, NCH], FP)
            with tc.high_priority():
                nc.scalar.dma_start(out=scl_sb[:], in_=scl)
                chunk_dma(nc.scalar, xtiles[1], 1)
                chunk_dma(nc.scalar, xtiles[3], 3)
                chunk_dma(nc.scalar, xtiles[5], 5)

            ohrep_sl = auxA_sb[0:BLOC, 0:WIN * K]
            valid_sl = auxA_sb[0:BLOC, WIN * K:WIN * K + WIN]
            I8_sl = auxA_sb[0:BLOC, WIN * K + WIN:WA]

            C = small.tile([NP, NCH], FP)
            nc.vector.memset(C[:], 0.0)

            # ---------- correction path Ln first (data arrives early);
            # its DVE ops are emitted after the window chain ----------
            lnG = small.tile([GROWS, GF], FP)
            nc.scalar.activation(out=lnG[:], in_=G[:], func=AF.Ln,
                                 bias=1.0, scale=-1.0)

            # ---------- big term: ONE fused ACT op per sub-chunk ----------
            def chunk_act(c):
                i, h = sub_loc[c]
                xb = xtiles[i]
                nc.scalar.activation(out=xb[:, h:h + S],
                                     in_=xb[:, h:h + S], func=AF.Ln,
                                     bias=1.0, scale=scl_sb[:, c:c + 1],
                                     accum_out=C[:, c:c + 1])

            for c in range(NCH - 2):
                chunk_act(c)

            # ---------- window path (ACT ops land before last chunks) ----
            nc.vector.tensor_tensor(out=Wp[:], in0=Wp[:], in1=ohrep_sl,
                                    op=ALU.mult)
            win_raw = small.tile([BLOC, WIN], FP)
            nc.vector.tensor_reduce(
                out=win_raw[:],
                in_=Wp[:].rearrange("b (i k) -> b i k", k=K),
                axis=AX.X, op=ALU.add)
            # positive term part 1: win_v = win_raw * valid
            winv = small.tile([BLOC, WIN], FP)
            nc.vector.tensor_tensor(out=winv[:], in0=win_raw[:],
                                    in1=valid_sl, op=ALU.mult)
            # exclusion: + sum_i valid * ln(1 - win_raw), fused
            expcol = small.tile([BLOC, 2], FP)
            lnw = small.tile([BLOC, WIN], FP)
            nc.scalar.activation(out=lnw[:], in_=win_raw[:], func=AF.Ln,
                                 bias=1.0, scale=-1.0)
            lnwv = small.tile([BLOC, WIN], FP)
            nc.vector.tensor_tensor(out=lnwv[:], in0=lnw[:], in1=valid_sl,
                                    op=ALU.mult)
            nc.vector.tensor_reduce(out=expcol[:, 0:1], in_=lnwv[:],
                                    axis=AX.X, op=ALU.add)
            # smoothed = win_v @ M (transpose first via identity)
            wvt_ps = psum.tile([WIN, BLOC], FP)
            nc.tensor.matmul(out=wvt_ps[:], lhsT=winv[:], rhs=I8_sl,
                             start=True, stop=True)
            wvt = small.tile([WIN, BLOC], FP)
            nc.vector.tensor_copy(out=wvt[:], in_=wvt_ps[:])
            sm_ps = psum.tile([BLOC, WIN], FP)
            nc.tensor.matmul(out=sm_ps[:], lhsT=wvt[:], rhs=auxM_sb[:],
                             start=True, stop=True)
            smc = small.tile([BLOC, WIN], FP)
            nc.vector.tensor_scalar(out=smc[:], in0=sm_ps[:],
                                    scalar1=EPS, scalar2=1.0,
                                    op0=ALU.max, op1=ALU.min)
            smv = small.tile([BLOC, WIN], FP)
            nc.vector.tensor_tensor(out=smv[:], in0=smc[:], in1=valid_sl,
                                    op=ALU.mult)
            mx = small.tile([BLOC, 1], FP)
            nc.vector.tensor_reduce(out=mx[:], in_=smv[:], axis=AX.X,
                                    op=ALU.max)
            # correction fold (DVE) after the window chain
            gcol = small.tile([GROWS, 1], FP)
            nc.vector.tensor_tensor(out=lnG[:], in0=lnG[:], in1=mcorr_sb[:],
                                    op=ALU.mult)
            nc.vector.tensor_reduce(out=gcol[:], in_=lnG[:], axis=AX.X,
                                    op=ALU.add)
            lnmx = small.tile([BLOC, 1], FP)
            nc.scalar.activation(out=lnmx[:], in_=mx[:], func=AF.Ln)
            nc.vector.tensor_scalar_mul(expcol[:, 1:2], lnmx[:], -1.0)

            # last two chunks after the window-path ACT ops
            for c in range(NCH - 2, NCH):
                chunk_act(c)

            # ---------- final partition reduce ----------
            negones = small.tile([NP, 1], FP)
            nc.vector.memset(negones[:], -1.0)
            ones24 = small.tile([GROWS, 1], FP)
            nc.vector.memset(ones24[:], 1.0)
            tot_ps = psum.tile([1, NCH + 3], FP)
            nc.tensor.matmul(out=tot_ps[:, 0:NCH], lhsT=negones[:], rhs=C[:],
                             start=True, stop=True)
            nc.tensor.matmul(out=tot_ps[:, NCH:NCH + 1],
                             lhsT=ones24[:], rhs=gcol[:],
                             start=True, stop=True)
            nc.tensor.matmul(out=tot_ps[:, NCH + 1:NCH + 3],
                             lhsT=ones24[0:BLOC, :], rhs=expcol[:],
                             start=True, stop=True)
            tot = small.tile([1, 1], FP)
            nc.vector.tensor_reduce(out=tot[:], in_=tot_ps[:], axis=AX.X,
                                    op=ALU.add)
            nc.sync.dma_start(out=outd, in_=tot[:])

    nc.compile()
    _NC_CACHE = nc
    return nc


def _make_in_maps(X, lengths, tgt, w_end):
    X = np.ascontiguousarray(np.asarray(X, dtype=np.float32))
    lengths = np.asarray(lengths, dtype=np.int64)
    tgt = np.asarray(tgt, dtype=np.int64)
    w_end = np.asarray(w_end, dtype=np.int64)

    tau_s = np.maximum(0, w_end + OFFSET_D - WIN)
    tau_e = np.minimum(tau_s + WIN, lengths)
    Lw = tau_e - tau_s

    Mmat = _conv_matrix()
    I8 = np.eye(BLOC, dtype=np.float32)

    in_maps = []
    for cr in range(NCORES):
        bs = slice(cr * BLOC, (cr + 1) * BLOC)
        ls, ts, lw, tg = lengths[bs], tau_s[bs], Lw[bs], tgt[bs]

        oh = np.zeros((BLOC, K), np.float32)
        oh[np.arange(BLOC), tg] = 1.0
        ohrep = np.broadcast_to(oh[:, None, :], (BLOC, WIN, K)) \
            .reshape(BLOC, WIN * K)
        valid8 = (np.arange(WIN)[None, :] < lw[:, None]).astype(np.float32)
        auxA = np.concatenate([ohrep, valid8, I8], axis=1)  # (8, WA)

        # per-partition-row valid-element counts: row p of batch b covers
        # flat [p*FROW, (p+1)*FROW) within the batch -> thr elems valid
        r = np.arange(NP)
        thr = np.clip(ls[r // RPB] * K - (r % RPB) * FROW, 0, FROW)  # (128,)
        # scl[p, c] = -1 if chunk c of row p is fully valid else 0
        cidx = np.arange(NCH)
        scl_arr = np.where(thr[:, None] >= (cidx[None, :] + 1) * S,
                           np.float32(-1.0), np.float32(0.0))

        # boundary corrections: per batch at most one row with 0<thr<FROW;
        # its partial chunk [c0*S, thr) is re-gathered 1600-aligned.
        coff_arr = np.zeros((GROWS, 1), np.int32)
        mcorr_arr = np.zeros((GROWS, GF), np.float32)
        for b in range(BLOC):
            rows = np.where((thr > 0) & (thr < FROW)
                            & (r // RPB == b))[0]
            if len(rows) == 0:
                continue
            p0 = int(rows[0])
            th = int(thr[p0])
            c0 = th // S
            L = th - c0 * S
            if L == 0:
                continue
            g = p0 * FROW + c0 * S          # global start elem of partial
            a = (g // GF) * GF              # aligned gather start
            for u in range(GR):
                coff_arr[GR * b + u, 0] = a // GF + u
                e = a + GF * u + np.arange(GF)      # global elem idx
                mcorr_arr[GR * b + u] = np.where(
                    (e >= g) & (e < g + L), np.float32(-1.0),
                    np.float32(0.0))

        gofs_arr = (np.arange(BLOC) * T + ts).astype(np.int32) \
            .reshape(BLOC, 1)
        offs_arr = np.zeros((GROWS, 2), np.int32)
        offs_arr[:, 0:1] = coff_arr
        offs_arr[0:BLOC, 1:2] = gofs_arr

        Xp = np.zeros(PADDED, np.float32)
        Xp[:NELEM] = X[bs].ravel()
        # chunk-major permutation: DMA chunk (a, b) of the (NP, FROW)
        # slab becomes the contiguous block [a*NP*S, b*NP*S)
        Xv_ = Xp[:NELEM].reshape(NP, FROW)
        Xcm = np.concatenate(
            [Xv_[:, a * S:b * S].ravel() for a, b in CHUNK_SPANS])
        in_maps.append({
            "Xs": Xcm,
            "Xg": Xp,
            "offs": offs_arr,
            "auxA": np.ascontiguousarray(auxA),
            "auxM": np.ascontiguousarray(Mmat),
            "mcorr": mcorr_arr,
            "scl": np.ascontiguousarray(scl_arr),
        })
    return in_maps


def kernel(X, lengths, tgt, w_end):
    nc = _build_program()
    in_maps = _make_in_maps(X, lengths, tgt, w_end)
    res = bass_utils.run_bass_kernel_spmd(
        nc, in_maps, core_ids=list(range(NCORES)))
    total = np.float32(0.0)
    for c in range(NCORES):
        total += np.float32(res.results[c]["out"][0, 0])
    return np.array(total, dtype=np.float32)
